# revision 5
# baseline (speedup 1.0000x reference)
"""GraphConv (DeepChem) Bass kernel for 8 Trainium2 NeuronCores.

Sharding: data-parallel over rows within each degree bucket. Each core owns
1/8 of every bucket (deg0: 1500 rows, deg1-10: 3750 rows each) plus a
replicated node_features table for gathers. W/b replicated.

Device algorithm per 128-row tile of degree d:
  - indirect-DMA gather one [128,128] tile per neighbor slot j
  - PE matmul-by-identity transposes each gathered tile, accumulating
    sum_j G_j^T into PSUM -> nbT [din, rows]
  - psum_outT = W[2d-1]^T @ nbT + W[2d]^T @ selfT  (self features arrive
    pre-transposed from the host shard prep)
  - DVE eviction adds bias (per-partition scalar) -> store outT slice
Host un-transposes and re-concatenates bucket shards.
"""
import os
import sys
import types
import numpy as np

import concourse.bass as bass
import concourse.bacc as bacc
import concourse.mybir as mybir
import concourse.tile as tile
from concourse.masks import make_identity
from concourse.bass_utils import run_bass_kernel_spmd

N_DEG0 = 12000
N_PER_DEG = 30000
MAX_DEG = 10
D = 128
N_NODES = N_DEG0 + MAX_DEG * N_PER_DEG  # 312000
N_PARAMS = 2 * MAX_DEG + 1  # 21
N_CORES = 8

C_DEG0 = N_DEG0 // N_CORES          # 1500
C_DEG = N_PER_DEG // N_CORES        # 3750
P_DEG0 = 1536                       # padded to 12 tiles of 128
P_DEG = 3840                        # padded to 30 tiles of 128
T_DEG0 = P_DEG0 // 128              # 12
T_DEG = P_DEG // 128                # 30
LOCAL_COLS = P_DEG0 + MAX_DEG * P_DEG  # 39936 local rows per core
N_GTILES = MAX_DEG * T_DEG          # 300 gather tiles per core

_COMPILED = None
LAST_RESULT = None


def _maybe_install_trace_hook():
    """Inject antenv.axon_hooks so trace=True can NTFF-profile under axon."""
    try:
        import antenv.axon_hooks  # noqa: F401
        return True
    except ImportError:
        pass
    try:
        hooks = types.ModuleType("antenv.axon_hooks")
        hooks._hook = None

        def _set(h):
            hooks._hook = h

        def _get():
            return hooks._hook

        hooks.set_axon_ntff_profile_hook = _set
        hooks.get_axon_ntff_profile_hook = _get
        sys.modules["antenv.axon_hooks"] = hooks
        import antenv

        antenv.axon_hooks = hooks
        from trn_agent_boot.trn_boot import _ntff_profile_via_ctypes

        _set(_ntff_profile_via_ctypes("/opt/axon/libaxon_pjrt.so"))
        return True
    except Exception:
        return False


def _build():
    nc = bacc.Bacc()
    nf = nc.declare_dram_parameter("nf", [N_NODES, D], mybir.dt.float32, isOutput=False)
    selfbT = nc.declare_dram_parameter(
        "selfbT", [D, LOCAL_COLS], mybir.dt.float32, isOutput=False
    )
    gidx = nc.declare_dram_parameter(
        "gidx", [128, N_GTILES * MAX_DEG], mybir.dt.int32, isOutput=False
    )
    w_in = nc.declare_dram_parameter(
        "w", [N_PARAMS, D, D], mybir.dt.float32, isOutput=False
    )
    bsumT = nc.declare_dram_parameter(
        "bsumT", [D, MAX_DEG + 1], mybir.dt.float32, isOutput=False
    )
    outT = nc.declare_dram_parameter(
        "outT", [D, LOCAL_COLS], mybir.dt.float32, isOutput=True
    )

    with tile.TileContext(nc) as tc:
        with (
            tc.tile_pool(name="const", bufs=1) as constp,
            tc.tile_pool(name="gp", bufs=14) as gp,
            tc.tile_pool(name="sfp", bufs=8) as sfp,
            tc.tile_pool(name="nbp", bufs=8) as nbp,
            tc.tile_pool(name="obp", bufs=8) as obp,
            tc.tile_pool(name="psnb", bufs=4, space="PSUM") as psnb,
            tc.tile_pool(name="psout", bufs=4, space="PSUM") as psout,
        ):
            identity = constp.tile([128, 128], mybir.dt.float32)
            make_identity(nc, identity[:])
            w_sb = constp.tile([128, N_PARAMS * 128], mybir.dt.float32)
            for k in range(N_PARAMS):
                nc.sync.dma_start(out=w_sb[:, k * 128:(k + 1) * 128], in_=w_in[k, :, :])
            bs_sb = constp.tile([128, MAX_DEG + 1], mybir.dt.float32)
            nc.sync.dma_start(out=bs_sb[:], in_=bsumT[:, :])
            ix_all = constp.tile([128, N_GTILES * MAX_DEG], mybir.dt.int32)
            nc.sync.dma_start(
                out=ix_all[:], in_=gidx[:, :]
            )

            def do_tile(d, col0, gtile):
                """One 128-row tile of degree d; local cols [col0, col0+128)."""
                sf = sfp.tile([128, 128], mybir.dt.float32, tag="sf")
                nc.sync.dma_start(out=sf[:], in_=selfbT[:, col0:col0 + 128])
                ps_o = psout.tile([128, 128], mybir.dt.float32, tag="pso")
                if d > 0:
                    g = gp.tile([128, d * 128], mybir.dt.float32, tag="g")
                    for j in range(d):
                        nc.gpsimd.indirect_dma_start(
                            out=g[:, j * 128:(j + 1) * 128],
                            out_offset=None,
                            in_=nf[:],
                            in_offset=bass.IndirectOffsetOnAxis(
                                ap=ix_all[:, gtile * MAX_DEG + j:gtile * MAX_DEG + j + 1],
                                axis=0,
                            ),
                        )
                    ps_nb = psnb.tile([128, 128], mybir.dt.float32, tag="psnb")
                    for j in range(d):
                        nc.tensor.matmul(
                            out=ps_nb[:],
                            lhsT=g[:, j * 128:(j + 1) * 128],
                            rhs=identity[:],
                            start=(j == 0),
                            stop=(j == d - 1),
                        )
                    nbT = nbp.tile([128, 128], mybir.dt.float32, tag="nb")
                    nc.vector.tensor_copy(out=nbT[:], in_=ps_nb[:])
                    nc.tensor.matmul(
                        out=ps_o[:],
                        lhsT=w_sb[:, (2 * d - 1) * 128:(2 * d) * 128],
                        rhs=nbT[:],
                        start=True,
                        stop=False,
                    )
                    nc.tensor.matmul(
                        out=ps_o[:],
                        lhsT=w_sb[:, (2 * d) * 128:(2 * d + 1) * 128],
                        rhs=sf[:],
                        start=False,
                        stop=True,
                    )
                else:
                    nc.tensor.matmul(
                        out=ps_o[:],
                        lhsT=w_sb[:, 0:128],
                        rhs=sf[:],
                        start=True,
                        stop=True,
                    )
                ob = obp.tile([128, 128], mybir.dt.float32, tag="ob")
                nc.vector.tensor_scalar_add(
                    out=ob[:], in0=ps_o[:], scalar1=bs_sb[:, d:d + 1]
                )
                nc.sync.dma_start(out=outT[:, col0:col0 + 128], in_=ob[:])

            for t in range(T_DEG0):
                do_tile(0, t * 128, -1)
            for t in range(T_DEG):
                for d in range(1, MAX_DEG + 1):
                    base = P_DEG0 + (d - 1) * P_DEG
                    do_tile(d, base + t * 128, (d - 1) * T_DEG + t)

    nc.compile()
    return nc


def kernel(node_features, deg_slice, adj1, adj2, adj3, adj4, adj5, adj6,
           adj7, adj8, adj9, adj10, W, b):
    global _COMPILED, LAST_RESULT
    nf = np.ascontiguousarray(np.asarray(node_features, dtype=np.float32))
    adjs = [np.asarray(a, dtype=np.int32)
            for a in (adj1, adj2, adj3, adj4, adj5, adj6, adj7, adj8, adj9, adj10)]
    Wf = np.asarray(W, dtype=np.float32)
    bf = np.asarray(b, dtype=np.float32)

    # bias pre-sum (affine marshalling): bsum[0]=b[0]; bsum[d]=b[2d-1]+b[2d]
    bsum = np.empty((MAX_DEG + 1, D), np.float32)
    bsum[0] = bf[0]
    for d in range(1, MAX_DEG + 1):
        bsum[d] = bf[2 * d - 1] + bf[2 * d]
    bsumT = np.ascontiguousarray(bsum.T)

    in_maps = []
    for c in range(N_CORES):
        selfb = np.zeros((LOCAL_COLS, D), np.float32)
        selfb[:C_DEG0] = nf[c * C_DEG0:(c + 1) * C_DEG0]
        gidx = np.zeros((N_GTILES, 128, MAX_DEG), np.int32)  # relaid below
        for d in range(1, MAX_DEG + 1):
            base = P_DEG0 + (d - 1) * P_DEG
            gs = N_DEG0 + (d - 1) * N_PER_DEG + c * C_DEG
            selfb[base:base + C_DEG] = nf[gs:gs + C_DEG]
            a = np.zeros((P_DEG, d), np.int32)
            a[:C_DEG] = adjs[d - 1][c * C_DEG:(c + 1) * C_DEG]
            gidx[(d - 1) * T_DEG:d * T_DEG, :, :d] = a.reshape(T_DEG, 128, d)
        in_maps.append({
            "nf": nf,
            "selfbT": np.ascontiguousarray(selfb.T),
            "gidx": np.ascontiguousarray(gidx.transpose(1, 0, 2).reshape(128, -1)),
            "w": Wf,
            "bsumT": bsumT,
        })

    if _COMPILED is None:
        _COMPILED = _build()

    trace = bool(int(os.environ.get("KERNEL_TRACE", "0")))
    if trace:
        trace = _maybe_install_trace_hook()
    res = run_bass_kernel_spmd(
        _COMPILED, in_maps, core_ids=list(range(N_CORES)), trace=trace
    )
    LAST_RESULT = res

    out = np.empty((N_NODES, D), np.float32)
    for c in range(N_CORES):
        oT = res.results[c]["outT"]
        out[c * C_DEG0:(c + 1) * C_DEG0] = oT[:, :C_DEG0].T
        for d in range(1, MAX_DEG + 1):
            base = P_DEG0 + (d - 1) * P_DEG
            gs = N_DEG0 + (d - 1) * N_PER_DEG + c * C_DEG
            out[gs:gs + C_DEG] = oT[:, base:base + C_DEG].T
    return out
